# revision 16
# baseline (speedup 1.0000x reference)
"""GAT layer (nn_GATlayer) on 8 Trainium2 NeuronCores via Bass/Tile.

Strategy (edge parallelism over dst-sorted edges, per the sharding hint):
  - Host: sort edges by dst, shard dst nodes contiguously across 8 cores
    (6250 nodes/core), partition each core's nodes into blocks of 128.
    Each block's edges are split by src parity (even/odd node id) so the
    node table can be indexed with int16 dma_gather indices (idx = src>>1
    into a stride-2-rows view of the table); each parity group is padded
    to TL/TH tiles of 128 edges (global maxima, so one program serves all
    cores). Host also casts the node table to bf16 (hb): a gathered edge
    row is 512B instead of 1KB.
  - Device, per chunk of CB=4 blocks: TWO dma_gather instructions (even +
    odd parity) fetch all the chunk's h[src] rows in one SWDGE descgen
    pass each (~1us fixed + 0.34ns/edge) — v5 issued one indirect DMA per
    128 edges, each costing ~1us of Pool-serialized descgen, which
    dominated the whole kernel.
  - Device, per block:
      * selection mask S_T[e, t*128+n] = (dst_local[e,t] == n) via DVE
        compare of host-prepped dst arrays against an iota constant
      * edge logit a = (slab_slot0 . w1) + b[dst]; the dst-side per-node
        dot b[n] = h_block[n,0:D] . w2 is broadcast to edges by
        transposing b on the PE and selecting through S_T; leaky_relu;
        w = exp(a) (no softmax max-shift: logits are O(5) and softmax is
        shift-invariant, fp32 exp handles it exactly)
      * PE matmuls S_w^T @ slab accumulate messages in PSUM across tiles;
        paired 1-col matmuls against a ones column accumulate the
        denominator s (the gathered slab must stay contiguous for
        dma_gather, so the ones column lives outside it); out = msg/s;
        empty nodes (s==0) keep h.
  - Padding edges gather node 0/1 (benign real data) and carry
    dst_local=SENT so their mask columns are all-zero: they contribute
    exactly nothing.
"""
import math

import numpy as np

import concourse.bacc as bacc
import concourse.bass as bass
import concourse.tile as tile
from concourse import mybir
from concourse.bass import AP
from concourse.bass_utils import run_bass_kernel_spmd

import bass_rust

# problem dims (overridable for small-scale sim tests)
CFG = dict(N=50000, S=2, D=128, n_cores=8)
SLOPE = 0.98
P = 128
SENT = 200.0
CB = 4  # blocks per gather chunk

_f32 = mybir.dt.float32
_i16 = mybir.dt.int16
_bf16 = mybir.dt.bfloat16


def _dims():
    n, s, d, ncores = CFG["N"], CFG["S"], CFG["D"], CFG["n_cores"]
    return n, s, d, s * d, ncores, n // ncores, math.ceil(n // ncores / P)


def _split_sync_waits(nc, max_waits=1):
    """walrus in this container allows only 1 sync wait per instruction;
    move overflow waits onto preceding nops on the same engine."""
    for f in nc.m.functions:
        for bb in f.blocks:
            insts = bb.instructions
            i = 0
            while i < len(insts):
                ins = insts[i]
                si = ins.sync_info
                if si is not None and len(si.on_wait) > max_waits:
                    waits = list(si.on_wait)
                    si.on_wait = waits[-max_waits:]
                    overflow = waits[:-max_waits]
                    eng = nc.engines[ins.engine]
                    nops = []
                    for j in range(0, len(overflow), max_waits):
                        nop = eng.nop(hint="split_wait", nofuse=True)
                        nop.ins.sync_info = bass_rust.SyncInfo(
                            on_wait=overflow[j : j + max_waits], on_update=[]
                        )
                        nops.append(nop.ins)
                    for f2 in nc.m.functions:
                        for bb2 in f2.blocks:
                            bb2.instructions[:] = [
                                x for x in bb2.instructions if x not in nops
                            ]
                    for k, nop_ins in enumerate(nops):
                        insts.insert(i + k, nop_ins)
                    i += len(nops)
                i += 1


def host_prep(src, dst):
    """Sort edges by dst; build per-core parity-split index arrays.

    Per block b the edges are split into even-src and odd-src groups,
    each padded to TL/TH tiles of 128 (global maxima). Edge slots:
      even tile t, slot p  ->  dst_cm col b*(TL+TH)+t,        partition p
      odd  tile t, slot p  ->  dst_cm col b*(TL+TH)+TL+t,     partition p
    dma_gather index order k = t*128+p matches (p, t) slab placement.
    idx arrays are int16, wrapped [16c + k%16, k//16] for the 8 Q7 cores.
    """
    n, s, d, sd, ncores, npc, nb = _dims()
    order = np.argsort(dst, kind="stable")
    s_src = np.ascontiguousarray(src[order]).astype(np.int64)
    s_dst = np.ascontiguousarray(dst[order]).astype(np.int64)

    groups = []  # (core, block) -> (lo_idx_array, hi_idx_array, dstl, dsth)
    for c in range(ncores):
        for b in range(nb):
            lo = c * npc + b * P
            hi = min(lo + P, (c + 1) * npc)
            e0 = np.searchsorted(s_dst, lo, side="left")
            e1 = np.searchsorted(s_dst, hi, side="left")
            es = s_src[e0:e1]
            ed = (s_dst[e0:e1] - lo).astype(np.float32)
            even = (es % 2) == 0
            groups.append((es[even] >> 1, (es[~even] - 1) >> 1,
                           ed[even], ed[~even]))
    TL = max(1, max(math.ceil(len(g[0]) / P) for g in groups))
    TH = max(1, max(math.ceil(len(g[1]) / P) for g in groups))

    TT = TL + TH
    per_core = []
    for c in range(ncores):
        dst_cm = np.full((P, nb * TT), SENT, np.float32)  # also used as fp32 mask_start
        kl = np.zeros(nb * TL * P, np.int16)
        kh = np.zeros(nb * TH * P, np.int16)
        for b in range(nb):
            gl, gh, dl, dh = groups[c * nb + b]
            kl[b * TL * P : b * TL * P + len(gl)] = gl
            kh[b * TH * P : b * TH * P + len(gh)] = gh
            dbuf = np.full(TL * P, SENT, np.float32)
            dbuf[: len(dl)] = dl
            dst_cm[:, b * TT : b * TT + TL] = dbuf.reshape(TL, P).T
            dbuf = np.full(TH * P, SENT, np.float32)
            dbuf[: len(dh)] = dh
            dst_cm[:, b * TT + TL : (b + 1) * TT] = dbuf.reshape(TH, P).T
        # 16-partition wrap, replicated into all 8 Q7 core windows
        idx_lo = np.zeros((P, nb * TL * P // 16), np.int16)
        idx_hi = np.zeros((P, nb * TH * P // 16), np.int16)
        wl = kl.reshape(-1, 16).T  # [16, cols]
        wh = kh.reshape(-1, 16).T
        for q in range(8):
            idx_lo[16 * q : 16 * q + 16, :] = wl
            idx_hi[16 * q : 16 * q + 16, :] = wh
        per_core.append(
            {"idx_lo": idx_lo, "idx_hi": idx_hi, "dst_cm": dst_cm}
        )
    return (TL, TH), per_core


def build_program(TLH, reps=1, mode="full"):
    """Build the SPMD Bass program (v7: chunked dma_gather, parity split)."""
    import ml_dtypes  # noqa: F401
    TL, TH = TLH
    n, s, d, sd, ncores, npc, nb = _dims()
    TT = TL + TH
    nhalf = n // 2
    nc = bacc.Bacc("TRN2", target_bir_lowering=False, debug=False,
                   num_devices=ncores, num_swdge_queues=2)
    hb_d = nc.dram_tensor("hb", [n, sd], _bf16, kind="ExternalInput").ap()
    wb_d = nc.dram_tensor("wb", [P, 2 * d], _f32, kind="ExternalInput").ap()
    w1b_d = nc.dram_tensor("w1b", [P, d], _bf16, kind="ExternalInput").ap()
    w1r_d = nc.dram_tensor("w1r", [P, (TL + TH) * d], _bf16,
                           kind="ExternalInput").ap()
    irb_d = nc.dram_tensor("irb", [P, P], _bf16, kind="ExternalInput").ap()
    id_d = nc.dram_tensor("id128", [P, P], _f32, kind="ExternalInput").ap()
    or_d = nc.dram_tensor("ones_row", [1, P], _f32, kind="ExternalInput").ap()
    il_d = nc.dram_tensor("idx_lo", [P, nb * TL * P // 16], _i16,
                          kind="ExternalInput").ap()
    ih_d = nc.dram_tensor("idx_hi", [P, nb * TH * P // 16], _i16,
                          kind="ExternalInput").ap()
    dstcm_d = nc.dram_tensor("dst_cm", [P, nb * TT], _bf16,
                             kind="ExternalInput").ap()
    hcm_d = nc.dram_tensor("hblk_cm", [P, nb * sd], _f32,
                           kind="ExternalInput").ap()
    y_d = nc.dram_tensor("y", [npc, sd], _f32, kind="ExternalOutput").ap()

    # parity views of the node table: row stride 2*sd, 256-elem rows
    hb_even = AP(hb_d.tensor, 0, [(2 * sd, nhalf), (1, sd)])
    hb_odd = AP(hb_d.tensor, sd, [(2 * sd, nhalf), (1, sd)])

    nchunks = math.ceil(nb / CB)
    CTMAX = CB * TT

    with tile.TileContext(nc) as tc:
        with (
            tc.tile_pool(name="const", bufs=1) as cpool,
            tc.tile_pool(name="psum", bufs=2, space="PSUM") as ppool,
        ):
            idx_lo = cpool.tile([P, nb * TL * P // 16], _i16)
            nc.sync.dma_start(out=idx_lo[:], in_=il_d[:])
            idx_hi = cpool.tile([P, nb * TH * P // 16], _i16)
            nc.sync.dma_start(out=idx_hi[:], in_=ih_d[:])
            dst_cm = cpool.tile([P, nb * TT], _bf16)
            nc.sync.dma_start(out=dst_cm[:], in_=dstcm_d[:])
            wb = cpool.tile([P, 2 * d], _f32)
            nc.sync.dma_start(out=wb[:], in_=wb_d[:])
            w1b = cpool.tile([P, d], _bf16)
            nc.sync.dma_start(out=w1b[:], in_=w1b_d[:])
            w1r = cpool.tile([P, TT * d], _bf16)
            nc.sync.dma_start(out=w1r[:], in_=w1r_d[:])
            irb = cpool.tile([P, P], _bf16)
            nc.sync.dma_start(out=irb[:], in_=irb_d[:])
            id128 = cpool.tile([P, P], _f32)
            nc.sync.dma_start(out=id128[:], in_=id_d[:])
            ones_row = cpool.tile([1, P], _f32)
            nc.sync.dma_start(out=ones_row[:], in_=or_d[:])
            hcm = cpool.tile([P, nb * sd], _f32)
            nc.sync.dma_start(out=hcm[:], in_=hcm_d[:])
            b_cols = cpool.tile([P, nb], _f32)
            ones_col = cpool.tile([P, 1], _bf16)
            nc.vector.memset(ones_col[:], 1.0)

            # phase 0: b[n] = h[n, 0:d] . w2 for all this core's nodes
            with tc.tile_pool(name="init", bufs=1) as ipool:
                bmul_all = ipool.tile([P, nb * d], _f32)
                hcm_ap = hcm[:]
                h0_v = AP(hcm_ap.tensor, hcm_ap.offset,
                          [hcm_ap.ap[0], (sd, nb), (1, d)])
                w2_sl = wb[:, d : 2 * d]
                w2_rep = AP(w2_sl.tensor, w2_sl.offset,
                            [w2_sl.ap[0], (0, nb), (1, d)])
                nc.vector.tensor_tensor(
                    out=bmul_all[:], in0=h0_v, in1=w2_rep,
                    op=mybir.AluOpType.mult,
                )
                bm = bmul_all[:]
                bm_v = AP(bm.tensor, bm.offset, [bm.ap[0], (d, nb), (1, d)])
                nc.vector.tensor_reduce(
                    out=b_cols[:], in_=bm_v, axis=mybir.AxisListType.X,
                    op=mybir.AluOpType.add,
                )
            with tc.tile_pool(name="work", bufs=2) as wpool:
                def stage_gather(ch):
                    """two dma_gathers for chunk ch; returns chunk slab."""
                    b0 = ch * CB
                    bcnt = min(CB, nb - b0)
                    slab = wpool.tile([P, CTMAX, sd], _bf16, tag="slab")
                    nlo = bcnt * TL * P
                    nc.gpsimd.dma_gather(
                        out_ap=slab[:, 0 : bcnt * TL, 0:sd],
                        in_ap=hb_even,
                        idxs_ap=idx_lo[:, b0 * TL * 8 : (b0 + bcnt) * TL * 8],
                        num_idxs=nlo,
                        num_idxs_reg=nlo,
                        elem_size=sd,
                        elem_step=2 * sd,
                        single_packet=False,
                    )
                    nhi = bcnt * TH * P
                    nc.gpsimd.dma_gather(
                        out_ap=slab[:, bcnt * TL : bcnt * TT, 0:sd],
                        in_ap=hb_odd,
                        idxs_ap=idx_hi[:, b0 * TH * 8 : (b0 + bcnt) * TH * 8],
                        num_idxs=nhi,
                        num_idxs_reg=nhi,
                        elem_size=sd,
                        elem_step=2 * sd,
                        single_packet=False,
                        queue_num=1,
                    )
                    return slab

                def slab_pos(ch, b, t):
                    """slab tile index of block b's tile t within chunk ch."""
                    b0 = ch * CB
                    bcnt = min(CB, nb - b0)
                    i = b - b0
                    if t < TL:
                        return i * TL + t
                    return bcnt * TL + i * TH + (t - TL)

                def stage_a(ch, b, slab):
                    """masks + logits + S_w for block b."""
                    g0 = b * TT
                    # S_T[e, t*P+n] = (dst_cm[e, g0+t] == n)
                    s_t = wpool.tile([P, TT * P], _bf16, tag="s_t")
                    dcm_sl = dst_cm[:, g0 : g0 + TT]
                    dcm_rep = AP(dcm_sl.tensor, dcm_sl.offset,
                                 dcm_sl.ap + [(0, P)])
                    ir_sl = irb[:, 0:P]
                    ir_rep = AP(ir_sl.tensor, ir_sl.offset,
                                [ir_sl.ap[0], (0, TT), (1, P)])
                    nc.vector.tensor_tensor(
                        out=s_t[:], in0=dcm_rep, in1=ir_rep,
                        op=mybir.AluOpType.is_equal,
                    )

                    # broadcast this block's b value row to all partitions
                    bt_ps = ppool.tile([1, P], _f32, tag="bt")
                    nc.tensor.matmul(out=bt_ps[:],
                                     lhsT=b_cols[:, b : b + 1],
                                     rhs=id128[:], start=True, stop=True)
                    b_row = wpool.tile([1, P], _f32, tag="b_row")
                    nc.vector.tensor_copy(out=b_row[:], in_=bt_ps[:])
                    bb_ps = ppool.tile([P, P], _f32, tag="bb")
                    nc.tensor.matmul(out=bb_ps[:], lhsT=ones_row[:],
                                     rhs=b_row[:], start=True, stop=True)
                    b_bc = wpool.tile([P, P], _bf16, tag="b_bc")
                    nc.vector.tensor_copy(out=b_bc[:], in_=bb_ps[:])

                    # select b[dst_e]: bsl = S_T * b_bc, b_mat = sum_n
                    bsl = wpool.tile([P, TT * P], _bf16, tag="bsl")
                    bb_sl = b_bc[:, 0:P]
                    bb_rep = AP(bb_sl.tensor, bb_sl.offset,
                                [bb_sl.ap[0], (0, TT), (1, P)])
                    nc.vector.tensor_tensor(
                        out=bsl[:], in0=s_t[:], in1=bb_rep,
                        op=mybir.AluOpType.mult,
                    )
                    b_mat = wpool.tile([P, TT], _f32, tag="b_mat")
                    bsl_ap = bsl[:]
                    bsl_v = AP(bsl_ap.tensor, bsl_ap.offset,
                               [bsl_ap.ap[0], (P, TT), (1, P)])
                    nc.vector.tensor_reduce(
                        out=b_mat[:], in_=bsl_v, axis=mybir.AxisListType.X,
                        op=mybir.AluOpType.add,
                    )

                    # src-side dot over the chunk slab's tiles of this block
                    dmul = wpool.tile([P, TT * d], _bf16, tag="dmul")
                    w1_sl = w1b[:, 0:d]
                    dot = wpool.tile([P, TT], _f32, tag="dot")
                    sl_ap = slab[:]
                    p_lo = slab_pos(ch, b, 0)
                    lo_v = AP(sl_ap.tensor, sl_ap.offset + p_lo * sd,
                              [sl_ap.ap[0], (sd, TL), (1, d)])
                    p_hi = slab_pos(ch, b, TL)
                    hi_v = AP(sl_ap.tensor, sl_ap.offset + p_hi * sd,
                              [sl_ap.ap[0], (sd, TH), (1, d)])
                    nc.vector.tensor_tensor(
                        out=dmul[:, 0 : TL * d], in0=lo_v,
                        in1=w1r[:, 0 : TL * d],
                        op=mybir.AluOpType.mult,
                    )
                    nc.vector.tensor_tensor(
                        out=dmul[:, TL * d : TT * d], in0=hi_v,
                        in1=w1r[:, 0 : TH * d],
                        op=mybir.AluOpType.mult,
                    )
                    dm = dmul[:]
                    dmul_v = AP(dm.tensor, dm.offset,
                                [dm.ap[0], (d, TT), (1, d)])
                    nc.vector.tensor_reduce(
                        out=dot[:], in_=dmul_v, axis=mybir.AxisListType.X,
                        op=mybir.AluOpType.add,
                    )

                    # a = dot + b ; leaky ; w = exp(a) (ACT, bf16 out)
                    a_mat = wpool.tile([P, TT], _f32, tag="a_mat")
                    nc.vector.tensor_tensor(
                        out=a_mat[:], in0=dot[:], in1=b_mat[:],
                        op=mybir.AluOpType.add,
                    )
                    a_sc = wpool.tile([P, TT], _f32, tag="a_sc")
                    nc.vector.tensor_scalar_mul(a_sc[:], a_mat[:], SLOPE)
                    nc.vector.tensor_tensor(
                        out=a_mat[:], in0=a_mat[:], in1=a_sc[:],
                        op=mybir.AluOpType.max,
                    )
                    w_mat = wpool.tile([P, TT], _bf16, tag="w_mat")
                    nc.scalar.activation(
                        out=w_mat[:], in_=a_mat[:],
                        func=mybir.ActivationFunctionType.Exp,
                    )

                    # S_w = S_T * w
                    wm_sl = w_mat[:, 0:TT]
                    wm_rep = AP(wm_sl.tensor, wm_sl.offset,
                                wm_sl.ap + [(0, P)])
                    nc.vector.tensor_tensor(
                        out=s_t[:], in0=s_t[:], in1=wm_rep,
                        op=mybir.AluOpType.mult,
                    )
                    return s_t

                def stage_mm(ch, b, slab, s_t):
                    """PE accumulation for block b; returns (acc, den)."""
                    acc = ppool.tile([P, sd], _f32, tag="acc")
                    den = ppool.tile([P, 1], _f32, tag="den")
                    for t in range(TT):
                        pos = slab_pos(ch, b, t)
                        lhsT = s_t[:, t * P : (t + 1) * P]
                        nc.tensor.matmul(
                            out=acc[:],
                            lhsT=lhsT,
                            rhs=slab[:, pos, 0:sd],
                            start=(t == 0), stop=(t == TT - 1),
                        )
                        nc.tensor.matmul(
                            out=den[:],
                            lhsT=lhsT,
                            rhs=ones_col[:, 0:1],
                            start=(t == 0), stop=(t == TT - 1),
                        )
                    return acc, den

                def stage_fin(b, acc, den):
                    """normalization + blend + store for block b."""
                    node_lo = b * P
                    nrows = min(P, npc - node_lo)
                    h_block = hcm[:, b * sd : (b + 1) * sd]
                    s_col = den[:, 0:1]

                    eq0 = wpool.tile([P, 1], _f32, tag="eq0")
                    nc.vector.tensor_scalar(
                        out=eq0[:], in0=s_col, scalar1=0.0,
                        scalar2=None, op0=mybir.AluOpType.is_equal,
                    )
                    s_safe = wpool.tile([P, 1], _f32, tag="s_safe")
                    nc.vector.tensor_tensor(
                        out=s_safe[:], in0=s_col, in1=eq0[:],
                        op=mybir.AluOpType.add,
                    )
                    rec = wpool.tile([P, 1], _f32, tag="rec")
                    nc.vector.reciprocal(out=rec[:], in_=s_safe[:])
                    out_sb = wpool.tile([P, sd], _f32, tag="out_sb")
                    nc.vector.tensor_scalar(
                        out=out_sb[:], in0=acc[:], scalar1=rec[:, 0:1],
                        scalar2=None, op0=mybir.AluOpType.mult,
                    )
                    hmask = wpool.tile([P, sd], _f32, tag="hmask")
                    nc.vector.tensor_scalar(
                        out=hmask[:], in0=h_block, scalar1=eq0[:, 0:1],
                        scalar2=None, op0=mybir.AluOpType.mult,
                    )
                    nc.vector.tensor_tensor(
                        out=out_sb[:], in0=out_sb[:], in1=hmask[:],
                        op=mybir.AluOpType.add,
                    )
                    nc.sync.dma_start(
                        out=y_d[:][node_lo : node_lo + nrows, :],
                        in_=out_sb[:nrows, :],
                    )

                def fake_gather(ch):
                    slab = wpool.tile([P, CTMAX, sd], _bf16, tag="slab")
                    nc.vector.memset(slab[:, 0:1, 0:sd], 1.0)
                    return slab

                def process_chunk(ch, slab):
                    b0, b1 = ch * CB, min((ch + 1) * CB, nb)
                    pend_a = None   # (b, s_t)
                    pend_mm = None  # (b, acc, den)
                    for b in range(b0, b1):
                        s_t = stage_a(ch, b, slab)
                        if pend_mm is not None:
                            stage_fin(pend_mm[0], pend_mm[1], pend_mm[2])
                            pend_mm = None
                        if pend_a is not None:
                            acc, den = stage_mm(ch, pend_a[0], slab,
                                                pend_a[1])
                            pend_mm = (pend_a[0], acc, den)
                        pend_a = (b, s_t)
                    acc, den = stage_mm(ch, pend_a[0], slab, pend_a[1])
                    if pend_mm is not None:
                        stage_fin(pend_mm[0], pend_mm[1], pend_mm[2])
                    stage_fin(pend_a[0], acc, den)

                for rep in range(reps):
                    prev = None  # (ch, slab)
                    for ch in range(nchunks):
                        if mode == "compute":
                            slab = fake_gather(ch)
                        else:
                            slab = stage_gather(ch)
                        if mode == "gather":
                            continue
                        if prev is not None:
                            process_chunk(prev[0], prev[1])
                        prev = (ch, slab)
                    if mode == "gather":
                        continue
                    process_chunk(prev[0], prev[1])

    nc.compile()
    _split_sync_waits(nc, max_waits=1)
    return nc


_cache = {}


def make_in_maps(h_features, w_att, per_core):
    import ml_dtypes
    bf16 = np.dtype(ml_dtypes.bfloat16)
    n, s, d, sd, ncores, npc, nb = _dims()
    TT = per_core[0]["dst_cm"].shape[1] // nb
    h2 = np.ascontiguousarray(h_features.reshape(n, sd), dtype=np.float32)
    hb = h2.astype(bf16)
    w_flat = np.ascontiguousarray(w_att.reshape(1, 2 * d), dtype=np.float32)
    wb = np.repeat(w_flat, P, axis=0)
    w1b = np.ascontiguousarray(wb[:, 0:d]).astype(bf16)
    TT = per_core[0]["dst_cm"].shape[1] // nb
    w1r = np.ascontiguousarray(np.tile(w1b, (1, TT)))
    irb = np.repeat(np.arange(P, dtype=np.float32).reshape(1, P), P,
                    axis=0).astype(bf16)
    id128 = np.eye(P, dtype=np.float32)
    ones_row = np.ones((1, P), np.float32)
    hcm_list = []
    for c in range(ncores):
        pad_rows = nb * P
        hp = np.zeros((pad_rows, sd), np.float32)
        hp[:npc] = h2[c * npc : (c + 1) * npc]
        hcm_list.append(
            np.ascontiguousarray(
                hp.reshape(nb, P, sd).transpose(1, 0, 2).reshape(P, nb * sd)
            )
        )
    in_maps = []
    for c in range(ncores):
        dst_cm = per_core[c]["dst_cm"]
        in_maps.append(
            {
                "hb": hb,
                "wb": wb,
                "w1b": w1b,
                "w1r": w1r,
                "irb": irb,
                "id128": id128,
                "ones_row": ones_row,
                "idx_lo": per_core[c]["idx_lo"],
                "idx_hi": per_core[c]["idx_hi"],
                "dst_cm": dst_cm.astype(bf16),
                "hblk_cm": hcm_list[c],
            }
        )
    return in_maps


def kernel(h_features, src, dst, w_att):
    n, s, d, sd, ncores, npc, nb = _dims()
    h_features = np.ascontiguousarray(h_features, dtype=np.float32)
    src = np.ascontiguousarray(src, dtype=np.int32)
    dst = np.ascontiguousarray(dst, dtype=np.int32)
    w_att = np.ascontiguousarray(w_att, dtype=np.float32)

    TLH, per_core = host_prep(src, dst)
    if TLH not in _cache:
        _cache[TLH] = build_program(TLH)
    nc = _cache[TLH]

    in_maps = make_in_maps(h_features, w_att, per_core)
    res = run_bass_kernel_spmd(nc, in_maps, list(range(ncores)))
    out = np.concatenate([res.results[c]["y"] for c in range(ncores)], axis=0)
    return out.reshape(n, s, d).astype(np.float32)


# revision 18
# speedup vs baseline: 1.0245x; 1.0245x over previous
"""GAT layer (nn_GATlayer) on 8 Trainium2 NeuronCores via Bass/Tile.

Strategy (edge parallelism over dst-sorted edges, per the sharding hint):
  - Host: sort edges by dst, shard dst nodes contiguously across 8 cores
    (6250 nodes/core), partition each core's nodes into blocks of 128.
    Each block's edges are split by src parity (even/odd node id) so the
    node table can be indexed with int16 dma_gather indices (idx = src>>1
    into a stride-2-rows view of the table); each parity group is padded
    to TL/TH tiles of 128 edges (global maxima, so one program serves all
    cores). Host also casts the node table to bf16 (hb): a gathered edge
    row is 512B instead of 1KB.
  - Device, per chunk of CB=4 blocks: TWO dma_gather instructions (even +
    odd parity) fetch all the chunk's h[src] rows in one SWDGE descgen
    pass each (~1us fixed + 0.34ns/edge) — v5 issued one indirect DMA per
    128 edges, each costing ~1us of Pool-serialized descgen, which
    dominated the whole kernel.
  - Device, per block:
      * selection mask S_T[e, t*128+n] = (dst_local[e,t] == n) via DVE
        compare of host-prepped dst arrays against an iota constant
      * edge logit a = (slab_slot0 . w1) + b[dst]; the dst-side per-node
        dot b[n] = h_block[n,0:D] . w2 is broadcast to edges by
        transposing b on the PE and selecting through S_T; leaky_relu;
        w = exp(a) (no softmax max-shift: logits are O(5) and softmax is
        shift-invariant, fp32 exp handles it exactly)
      * PE matmuls S_w^T @ slab accumulate messages in PSUM across tiles;
        paired 1-col matmuls against a ones column accumulate the
        denominator s (the gathered slab must stay contiguous for
        dma_gather, so the ones column lives outside it); out = msg/s;
        empty nodes (s==0) keep h.
  - Padding edges gather node 0/1 (benign real data) and carry
    dst_local=SENT so their mask columns are all-zero: they contribute
    exactly nothing.
"""
import math

import numpy as np

import concourse.bacc as bacc
import concourse.bass as bass
import concourse.tile as tile
from concourse import mybir
from concourse.bass import AP
from concourse.bass_utils import run_bass_kernel_spmd

import bass_rust

# problem dims (overridable for small-scale sim tests)
CFG = dict(N=50000, S=2, D=128, n_cores=8)
SLOPE = 0.98
P = 128
SENT = 200.0
CB = 4  # blocks per gather chunk

_f32 = mybir.dt.float32
_i16 = mybir.dt.int16
_bf16 = mybir.dt.bfloat16


def _dims():
    n, s, d, ncores = CFG["N"], CFG["S"], CFG["D"], CFG["n_cores"]
    return n, s, d, s * d, ncores, n // ncores, math.ceil(n // ncores / P)


def _split_sync_waits(nc, max_waits=1):
    """walrus in this container allows only 1 sync wait per instruction;
    move overflow waits onto preceding nops on the same engine."""
    for f in nc.m.functions:
        for bb in f.blocks:
            insts = bb.instructions
            i = 0
            while i < len(insts):
                ins = insts[i]
                si = ins.sync_info
                if si is not None and len(si.on_wait) > max_waits:
                    waits = list(si.on_wait)
                    si.on_wait = waits[-max_waits:]
                    overflow = waits[:-max_waits]
                    eng = nc.engines[ins.engine]
                    nops = []
                    for j in range(0, len(overflow), max_waits):
                        nop = eng.nop(hint="split_wait", nofuse=True)
                        nop.ins.sync_info = bass_rust.SyncInfo(
                            on_wait=overflow[j : j + max_waits], on_update=[]
                        )
                        nops.append(nop.ins)
                    for f2 in nc.m.functions:
                        for bb2 in f2.blocks:
                            bb2.instructions[:] = [
                                x for x in bb2.instructions if x not in nops
                            ]
                    for k, nop_ins in enumerate(nops):
                        insts.insert(i + k, nop_ins)
                    i += len(nops)
                i += 1


def host_prep(src, dst):
    """Sort edges by dst; build per-core parity-split index arrays.

    Per block b the edges are split into even-src and odd-src groups,
    each padded to TL/TH tiles of 128 (global maxima). Edge slots:
      even tile t, slot p  ->  dst_cm col b*(TL+TH)+t,        partition p
      odd  tile t, slot p  ->  dst_cm col b*(TL+TH)+TL+t,     partition p
    dma_gather index order k = t*128+p matches (p, t) slab placement.
    idx arrays are int16, wrapped [16c + k%16, k//16] for the 8 Q7 cores.
    """
    n, s, d, sd, ncores, npc, nb = _dims()
    order = np.argsort(dst, kind="stable")
    s_src = np.ascontiguousarray(src[order]).astype(np.int64)
    s_dst = np.ascontiguousarray(dst[order]).astype(np.int64)

    groups = []  # (core, block) -> (lo_idx_array, hi_idx_array, dstl, dsth)
    for c in range(ncores):
        for b in range(nb):
            lo = c * npc + b * P
            hi = min(lo + P, (c + 1) * npc)
            e0 = np.searchsorted(s_dst, lo, side="left")
            e1 = np.searchsorted(s_dst, hi, side="left")
            es = s_src[e0:e1]
            ed = (s_dst[e0:e1] - lo).astype(np.float32)
            even = (es % 2) == 0
            groups.append((es[even] >> 1, (es[~even] - 1) >> 1,
                           ed[even], ed[~even]))
    TL = max(1, max(math.ceil(len(g[0]) / P) for g in groups))
    TH = max(1, max(math.ceil(len(g[1]) / P) for g in groups))

    TT = TL + TH
    per_core = []
    for c in range(ncores):
        dst_cm = np.full((P, nb * TT), SENT, np.float32)  # also used as fp32 mask_start
        kl = np.zeros(nb * TL * P, np.int16)
        kh = np.zeros(nb * TH * P, np.int16)
        for b in range(nb):
            gl, gh, dl, dh = groups[c * nb + b]
            kl[b * TL * P : b * TL * P + len(gl)] = gl
            kh[b * TH * P : b * TH * P + len(gh)] = gh
            dbuf = np.full(TL * P, SENT, np.float32)
            dbuf[: len(dl)] = dl
            dst_cm[:, b * TT : b * TT + TL] = dbuf.reshape(TL, P).T
            dbuf = np.full(TH * P, SENT, np.float32)
            dbuf[: len(dh)] = dh
            dst_cm[:, b * TT + TL : (b + 1) * TT] = dbuf.reshape(TH, P).T
        # 16-partition wrap, replicated into all 8 Q7 core windows
        idx_lo = np.zeros((P, nb * TL * P // 16), np.int16)
        idx_hi = np.zeros((P, nb * TH * P // 16), np.int16)
        wl = kl.reshape(-1, 16).T  # [16, cols]
        wh = kh.reshape(-1, 16).T
        for q in range(8):
            idx_lo[16 * q : 16 * q + 16, :] = wl
            idx_hi[16 * q : 16 * q + 16, :] = wh
        per_core.append(
            {"idx_lo": idx_lo, "idx_hi": idx_hi, "dst_cm": dst_cm}
        )
    return (TL, TH), per_core


def build_program(TLH, reps=1, mode="full"):
    """Build the SPMD Bass program (v7: chunked dma_gather, parity split)."""
    import ml_dtypes  # noqa: F401
    TL, TH = TLH
    n, s, d, sd, ncores, npc, nb = _dims()
    TT = TL + TH
    nhalf = n // 2
    nc = bacc.Bacc("TRN2", target_bir_lowering=False, debug=False,
                   num_devices=ncores)
    hb_d = nc.dram_tensor("hb", [n, sd], _bf16, kind="ExternalInput").ap()
    wb_d = nc.dram_tensor("wb", [P, 2 * d], _f32, kind="ExternalInput").ap()
    w1b_d = nc.dram_tensor("w1b", [P, d], _bf16, kind="ExternalInput").ap()
    w1r_d = nc.dram_tensor("w1r", [P, (TL + TH) * d], _bf16,
                           kind="ExternalInput").ap()
    irb_d = nc.dram_tensor("irb", [P, P], _bf16, kind="ExternalInput").ap()
    id_d = nc.dram_tensor("id128", [P, P], _f32, kind="ExternalInput").ap()
    or_d = nc.dram_tensor("ones_row", [1, P], _f32, kind="ExternalInput").ap()
    il_d = nc.dram_tensor("idx_lo", [P, nb * TL * P // 16], _i16,
                          kind="ExternalInput").ap()
    ih_d = nc.dram_tensor("idx_hi", [P, nb * TH * P // 16], _i16,
                          kind="ExternalInput").ap()
    dstcm_d = nc.dram_tensor("dst_cm", [P, nb * TT], _bf16,
                             kind="ExternalInput").ap()
    hcm_d = nc.dram_tensor("hblk_cm", [P, nb * sd], _f32,
                           kind="ExternalInput").ap()
    y_d = nc.dram_tensor("y", [npc, sd], _f32, kind="ExternalOutput").ap()

    # parity views of the node table: row stride 2*sd, 256-elem rows
    hb_even = AP(hb_d.tensor, 0, [(2 * sd, nhalf), (1, sd)])
    hb_odd = AP(hb_d.tensor, sd, [(2 * sd, nhalf), (1, sd)])

    nchunks = math.ceil(nb / CB)
    CTMAX = CB * TT

    with tile.TileContext(nc) as tc:
        with (
            tc.tile_pool(name="const", bufs=1) as cpool,
            tc.tile_pool(name="psum", bufs=2, space="PSUM") as ppool,
        ):
            idx_lo = cpool.tile([P, nb * TL * P // 16], _i16)
            nc.sync.dma_start(out=idx_lo[:], in_=il_d[:])
            idx_hi = cpool.tile([P, nb * TH * P // 16], _i16)
            nc.sync.dma_start(out=idx_hi[:], in_=ih_d[:])
            dst_cm = cpool.tile([P, nb * TT], _bf16)
            nc.sync.dma_start(out=dst_cm[:], in_=dstcm_d[:])
            wb = cpool.tile([P, 2 * d], _f32)
            nc.sync.dma_start(out=wb[:], in_=wb_d[:])
            w1b = cpool.tile([P, d], _bf16)
            nc.sync.dma_start(out=w1b[:], in_=w1b_d[:])
            w1r = cpool.tile([P, TT * d], _bf16)
            nc.sync.dma_start(out=w1r[:], in_=w1r_d[:])
            irb = cpool.tile([P, P], _bf16)
            nc.sync.dma_start(out=irb[:], in_=irb_d[:])
            id128 = cpool.tile([P, P], _f32)
            nc.sync.dma_start(out=id128[:], in_=id_d[:])
            ones_row = cpool.tile([1, P], _f32)
            nc.sync.dma_start(out=ones_row[:], in_=or_d[:])
            hcm = cpool.tile([P, nb * sd], _f32)
            nc.sync.dma_start(out=hcm[:], in_=hcm_d[:])
            b_cols = cpool.tile([P, nb], _f32)
            ones_col = cpool.tile([P, 1], _bf16)
            nc.vector.memset(ones_col[:], 1.0)

            # phase 0: b[n] = h[n, 0:d] . w2 for all this core's nodes
            with tc.tile_pool(name="init", bufs=1) as ipool:
                bmul_all = ipool.tile([P, nb * d], _f32)
                hcm_ap = hcm[:]
                h0_v = AP(hcm_ap.tensor, hcm_ap.offset,
                          [hcm_ap.ap[0], (sd, nb), (1, d)])
                w2_sl = wb[:, d : 2 * d]
                w2_rep = AP(w2_sl.tensor, w2_sl.offset,
                            [w2_sl.ap[0], (0, nb), (1, d)])
                nc.vector.tensor_tensor(
                    out=bmul_all[:], in0=h0_v, in1=w2_rep,
                    op=mybir.AluOpType.mult,
                )
                bm = bmul_all[:]
                bm_v = AP(bm.tensor, bm.offset, [bm.ap[0], (d, nb), (1, d)])
                nc.vector.tensor_reduce(
                    out=b_cols[:], in_=bm_v, axis=mybir.AxisListType.X,
                    op=mybir.AluOpType.add,
                )
            with tc.tile_pool(name="work", bufs=2) as wpool:
                def stage_gather(ch):
                    """two dma_gathers for chunk ch; returns chunk slab."""
                    b0 = ch * CB
                    bcnt = min(CB, nb - b0)
                    slab = wpool.tile([P, CTMAX, sd], _bf16, tag="slab")
                    nlo = bcnt * TL * P
                    nc.gpsimd.dma_gather(
                        out_ap=slab[:, 0 : bcnt * TL, 0:sd],
                        in_ap=hb_even,
                        idxs_ap=idx_lo[:, b0 * TL * 8 : (b0 + bcnt) * TL * 8],
                        num_idxs=nlo,
                        num_idxs_reg=nlo,
                        elem_size=sd,
                        elem_step=2 * sd,
                        single_packet=False,
                    )
                    nhi = bcnt * TH * P
                    nc.gpsimd.dma_gather(
                        out_ap=slab[:, bcnt * TL : bcnt * TT, 0:sd],
                        in_ap=hb_odd,
                        idxs_ap=idx_hi[:, b0 * TH * 8 : (b0 + bcnt) * TH * 8],
                        num_idxs=nhi,
                        num_idxs_reg=nhi,
                        elem_size=sd,
                        elem_step=2 * sd,
                        single_packet=False,
                    )
                    return slab

                def slab_pos(ch, b, t):
                    """slab tile index of block b's tile t within chunk ch."""
                    b0 = ch * CB
                    bcnt = min(CB, nb - b0)
                    i = b - b0
                    if t < TL:
                        return i * TL + t
                    return bcnt * TL + i * TH + (t - TL)

                def stage_a(ch, b, slab):
                    """masks + logits + S_w for block b."""
                    g0 = b * TT
                    # S_T[e, t*P+n] = (dst_cm[e, g0+t] == n)
                    s_t = wpool.tile([P, TT * P], _bf16, tag="s_t")
                    dcm_sl = dst_cm[:, g0 : g0 + TT]
                    dcm_rep = AP(dcm_sl.tensor, dcm_sl.offset,
                                 dcm_sl.ap + [(0, P)])
                    ir_sl = irb[:, 0:P]
                    ir_rep = AP(ir_sl.tensor, ir_sl.offset,
                                [ir_sl.ap[0], (0, TT), (1, P)])
                    nc.vector.tensor_tensor(
                        out=s_t[:], in0=dcm_rep, in1=ir_rep,
                        op=mybir.AluOpType.is_equal,
                    )

                    # broadcast this block's b value row to all partitions
                    bt_ps = ppool.tile([1, P], _f32, tag="bt")
                    nc.tensor.matmul(out=bt_ps[:],
                                     lhsT=b_cols[:, b : b + 1],
                                     rhs=id128[:], start=True, stop=True)
                    b_row = wpool.tile([1, P], _f32, tag="b_row")
                    nc.vector.tensor_copy(out=b_row[:], in_=bt_ps[:])
                    bb_ps = ppool.tile([P, P], _f32, tag="bb")
                    nc.tensor.matmul(out=bb_ps[:], lhsT=ones_row[:],
                                     rhs=b_row[:], start=True, stop=True)
                    b_bc = wpool.tile([P, P], _bf16, tag="b_bc")
                    nc.vector.tensor_copy(out=b_bc[:], in_=bb_ps[:])

                    # select b[dst_e]: bsl = S_T * b_bc, b_mat = sum_n
                    bsl = wpool.tile([P, TT * P], _bf16, tag="bsl")
                    bb_sl = b_bc[:, 0:P]
                    bb_rep = AP(bb_sl.tensor, bb_sl.offset,
                                [bb_sl.ap[0], (0, TT), (1, P)])
                    nc.vector.tensor_tensor(
                        out=bsl[:], in0=s_t[:], in1=bb_rep,
                        op=mybir.AluOpType.mult,
                    )
                    b_mat = wpool.tile([P, TT], _f32, tag="b_mat")
                    bsl_ap = bsl[:]
                    bsl_v = AP(bsl_ap.tensor, bsl_ap.offset,
                               [bsl_ap.ap[0], (P, TT), (1, P)])
                    nc.vector.tensor_reduce(
                        out=b_mat[:], in_=bsl_v, axis=mybir.AxisListType.X,
                        op=mybir.AluOpType.add,
                    )

                    # src-side dot over the chunk slab's tiles of this block
                    dmul = wpool.tile([P, TT * d], _bf16, tag="dmul")
                    w1_sl = w1b[:, 0:d]
                    dot = wpool.tile([P, TT], _f32, tag="dot")
                    sl_ap = slab[:]
                    p_lo = slab_pos(ch, b, 0)
                    lo_v = AP(sl_ap.tensor, sl_ap.offset + p_lo * sd,
                              [sl_ap.ap[0], (sd, TL), (1, d)])
                    p_hi = slab_pos(ch, b, TL)
                    hi_v = AP(sl_ap.tensor, sl_ap.offset + p_hi * sd,
                              [sl_ap.ap[0], (sd, TH), (1, d)])
                    nc.vector.tensor_tensor(
                        out=dmul[:, 0 : TL * d], in0=lo_v,
                        in1=w1r[:, 0 : TL * d],
                        op=mybir.AluOpType.mult,
                    )
                    nc.vector.tensor_tensor(
                        out=dmul[:, TL * d : TT * d], in0=hi_v,
                        in1=w1r[:, 0 : TH * d],
                        op=mybir.AluOpType.mult,
                    )
                    dm = dmul[:]
                    dmul_v = AP(dm.tensor, dm.offset,
                                [dm.ap[0], (d, TT), (1, d)])
                    nc.vector.tensor_reduce(
                        out=dot[:], in_=dmul_v, axis=mybir.AxisListType.X,
                        op=mybir.AluOpType.add,
                    )

                    # a = dot + b ; leaky ; w = exp(a) (ACT, bf16 out)
                    a_mat = wpool.tile([P, TT], _f32, tag="a_mat")
                    nc.vector.tensor_tensor(
                        out=a_mat[:], in0=dot[:], in1=b_mat[:],
                        op=mybir.AluOpType.add,
                    )
                    a_sc = wpool.tile([P, TT], _f32, tag="a_sc")
                    nc.vector.tensor_scalar_mul(a_sc[:], a_mat[:], SLOPE)
                    nc.vector.tensor_tensor(
                        out=a_mat[:], in0=a_mat[:], in1=a_sc[:],
                        op=mybir.AluOpType.max,
                    )
                    w_mat = wpool.tile([P, TT], _bf16, tag="w_mat")
                    nc.scalar.activation(
                        out=w_mat[:], in_=a_mat[:],
                        func=mybir.ActivationFunctionType.Exp,
                    )

                    # S_w = S_T * w
                    wm_sl = w_mat[:, 0:TT]
                    wm_rep = AP(wm_sl.tensor, wm_sl.offset,
                                wm_sl.ap + [(0, P)])
                    nc.vector.tensor_tensor(
                        out=s_t[:], in0=s_t[:], in1=wm_rep,
                        op=mybir.AluOpType.mult,
                    )
                    return s_t

                def stage_mm(ch, b, slab, s_t):
                    """PE accumulation for block b; returns (acc, den)."""
                    acc = ppool.tile([P, sd], _f32, tag="acc")
                    den = ppool.tile([P, 1], _f32, tag="den")
                    for t in range(TT):
                        pos = slab_pos(ch, b, t)
                        lhsT = s_t[:, t * P : (t + 1) * P]
                        nc.tensor.matmul(
                            out=acc[:],
                            lhsT=lhsT,
                            rhs=slab[:, pos, 0:sd],
                            start=(t == 0), stop=(t == TT - 1),
                        )
                        nc.tensor.matmul(
                            out=den[:],
                            lhsT=lhsT,
                            rhs=ones_col[:, 0:1],
                            start=(t == 0), stop=(t == TT - 1),
                        )
                    return acc, den

                def stage_fin(b, acc, den):
                    """normalization + blend + store for block b."""
                    node_lo = b * P
                    nrows = min(P, npc - node_lo)
                    h_block = hcm[:, b * sd : (b + 1) * sd]
                    s_col = den[:, 0:1]

                    eq0 = wpool.tile([P, 1], _f32, tag="eq0")
                    nc.vector.tensor_scalar(
                        out=eq0[:], in0=s_col, scalar1=0.0,
                        scalar2=None, op0=mybir.AluOpType.is_equal,
                    )
                    s_safe = wpool.tile([P, 1], _f32, tag="s_safe")
                    nc.vector.tensor_tensor(
                        out=s_safe[:], in0=s_col, in1=eq0[:],
                        op=mybir.AluOpType.add,
                    )
                    rec = wpool.tile([P, 1], _f32, tag="rec")
                    nc.vector.reciprocal(out=rec[:], in_=s_safe[:])
                    out_sb = wpool.tile([P, sd], _f32, tag="out_sb")
                    nc.vector.tensor_scalar(
                        out=out_sb[:], in0=acc[:], scalar1=rec[:, 0:1],
                        scalar2=None, op0=mybir.AluOpType.mult,
                    )
                    hmask = wpool.tile([P, sd], _f32, tag="hmask")
                    nc.vector.tensor_scalar(
                        out=hmask[:], in0=h_block, scalar1=eq0[:, 0:1],
                        scalar2=None, op0=mybir.AluOpType.mult,
                    )
                    nc.vector.tensor_tensor(
                        out=out_sb[:], in0=out_sb[:], in1=hmask[:],
                        op=mybir.AluOpType.add,
                    )
                    nc.sync.dma_start(
                        out=y_d[:][node_lo : node_lo + nrows, :],
                        in_=out_sb[:nrows, :],
                    )

                def fake_gather(ch):
                    slab = wpool.tile([P, CTMAX, sd], _bf16, tag="slab")
                    nc.vector.memset(slab[:, 0:1, 0:sd], 1.0)
                    return slab

                def process_chunk(ch, slab):
                    b0, b1 = ch * CB, min((ch + 1) * CB, nb)
                    pend_a = None   # (b, s_t)
                    pend_mm = None  # (b, acc, den)
                    for b in range(b0, b1):
                        s_t = stage_a(ch, b, slab)
                        if pend_mm is not None:
                            stage_fin(pend_mm[0], pend_mm[1], pend_mm[2])
                            pend_mm = None
                        if pend_a is not None:
                            acc, den = stage_mm(ch, pend_a[0], slab,
                                                pend_a[1])
                            pend_mm = (pend_a[0], acc, den)
                        pend_a = (b, s_t)
                    acc, den = stage_mm(ch, pend_a[0], slab, pend_a[1])
                    if pend_mm is not None:
                        stage_fin(pend_mm[0], pend_mm[1], pend_mm[2])
                    stage_fin(pend_a[0], acc, den)

                for rep in range(reps):
                    prev = None  # (ch, slab)
                    for ch in range(nchunks):
                        if mode == "compute":
                            slab = fake_gather(ch)
                        else:
                            slab = stage_gather(ch)
                        if mode == "gather":
                            continue
                        if prev is not None:
                            process_chunk(prev[0], prev[1])
                        prev = (ch, slab)
                    if mode == "gather":
                        continue
                    process_chunk(prev[0], prev[1])

    nc.compile()
    _split_sync_waits(nc, max_waits=1)
    return nc


_cache = {}


def make_in_maps(h_features, w_att, per_core):
    import ml_dtypes
    bf16 = np.dtype(ml_dtypes.bfloat16)
    n, s, d, sd, ncores, npc, nb = _dims()
    TT = per_core[0]["dst_cm"].shape[1] // nb
    h2 = np.ascontiguousarray(h_features.reshape(n, sd), dtype=np.float32)
    hb = h2.astype(bf16)
    w_flat = np.ascontiguousarray(w_att.reshape(1, 2 * d), dtype=np.float32)
    wb = np.repeat(w_flat, P, axis=0)
    w1b = np.ascontiguousarray(wb[:, 0:d]).astype(bf16)
    TTr = per_core[0]["dst_cm"].shape[1] // nb
    w1r = np.ascontiguousarray(np.tile(w1b, (1, TTr)))
    irb = np.repeat(np.arange(P, dtype=np.float32).reshape(1, P), P,
                    axis=0).astype(bf16)
    id128 = np.eye(P, dtype=np.float32)
    ones_row = np.ones((1, P), np.float32)
    hcm_list = []
    for c in range(ncores):
        pad_rows = nb * P
        hp = np.zeros((pad_rows, sd), np.float32)
        hp[:npc] = h2[c * npc : (c + 1) * npc]
        hcm_list.append(
            np.ascontiguousarray(
                hp.reshape(nb, P, sd).transpose(1, 0, 2).reshape(P, nb * sd)
            )
        )
    in_maps = []
    for c in range(ncores):
        dst_cm = per_core[c]["dst_cm"]
        in_maps.append(
            {
                "hb": hb,
                "wb": wb,
                "w1b": w1b,
                "w1r": w1r,
                "irb": irb,
                "id128": id128,
                "ones_row": ones_row,
                "idx_lo": per_core[c]["idx_lo"],
                "idx_hi": per_core[c]["idx_hi"],
                "dst_cm": dst_cm.astype(bf16),
                "hblk_cm": hcm_list[c],
            }
        )
    return in_maps


def kernel(h_features, src, dst, w_att):
    n, s, d, sd, ncores, npc, nb = _dims()
    h_features = np.ascontiguousarray(h_features, dtype=np.float32)
    src = np.ascontiguousarray(src, dtype=np.int32)
    dst = np.ascontiguousarray(dst, dtype=np.int32)
    w_att = np.ascontiguousarray(w_att, dtype=np.float32)

    TLH, per_core = host_prep(src, dst)
    if TLH not in _cache:
        _cache[TLH] = build_program(TLH)
    nc = _cache[TLH]

    in_maps = make_in_maps(h_features, w_att, per_core)
    res = run_bass_kernel_spmd(nc, in_maps, list(range(ncores)))
    out = np.concatenate([res.results[c]["y"] for c in range(ncores)], axis=0)
    return out.reshape(n, s, d).astype(np.float32)
